# revision 16
# baseline (speedup 1.0000x reference)
"""Trainium2 Bass kernel for nn_DenseCondenser (TT contraction, 65536x4096 -> 65536x8).

The three (8,8,8) TT cores compose into a single effective matrix E (4096, 8)
(the whole map is linear in x), folded on host in float64. The device kernel
is then a memory-bound skinny matmul out = x @ E + bias, data-parallel over
the batch across 8 NeuronCores.

Device-side layout: x is staged per-core as xT (4096, 8192) so the
contraction dim lands on SBUF partitions (TensorE contracts over partitions).
Each core streams 16 chunks of (128 part, 32 ktiles, 512 batch) fp32, runs 32
accumulating matmuls per chunk with the tiny E k-tile (128, 8) as the
stationary operand, adds bias on ScalarE, and stores out.T (8, 8192).
"""

import numpy as np

import concourse.bass as bass
import concourse.mybir as mybir
import concourse.tile as tile
from concourse import bacc
from concourse.bass import ts
from concourse.bass_utils import run_bass_kernel_spmd

# Problem shapes (hardcoded per harness contract)
BATCH = 65536
K = 4096  # input features = 8**4
C = 8  # output features
N_CORES = 8
B_CORE = BATCH // N_CORES  # 8192
CHUNK = 512  # batch columns per matmul (max fp32 moving free dim)
NK = K // 128  # 32 k-tiles
NCHUNK = B_CORE // CHUNK  # 16

# "fp32" = exact (4 cyc/col), "fp32r" = fast PE mode (1 cyc/col at N>=256)
MODE = "fp32r"

_program_cache = {}


def _build_program(mode: str) -> bass.Bass:
    f32 = mybir.dt.float32
    # In fp32r mode the x/E tensors are declared float32r end-to-end; the
    # host pre-rounds them to e8m11 (what fp32r is) so HW/verifier agree.
    mmdt = mybir.dt.float32r if mode == "fp32r" else f32
    nc = bacc.Bacc(None, name="dense_condenser")

    # xb[j, p, kt, b] = x[j*CHUNK + b, kt*128 + p]: per (chunk, partition)
    # the (kt, b) payload is one contiguous 64 KiB run -> max DMA efficiency.
    xb = nc.dram_tensor("xb", (NCHUNK, 128, NK, CHUNK), mmdt, kind="ExternalInput")
    eb = nc.dram_tensor("eb", (128, NK, C), mmdt, kind="ExternalInput")
    bias = nc.dram_tensor("bias", (C, 1), f32, kind="ExternalInput")
    outT = nc.dram_tensor("outT", (C, B_CORE), f32, kind="ExternalOutput")

    with tile.TileContext(nc) as tc:
        with (
            tc.tile_pool(name="consts", bufs=1) as consts,
            tc.tile_pool(name="xp", bufs=2) as xp,
            tc.tile_pool(name="op", bufs=2) as op,
            tc.tile_pool(name="pp", bufs=2, space=bass.MemorySpace.PSUM) as pp,
        ):
            e_tile = consts.tile([128, NK, C], mmdt)
            bias_tile = consts.tile([C, 1], f32)
            nc.sync.dma_start(out=e_tile[:], in_=eb[:])
            nc.sync.dma_start(out=bias_tile[:], in_=bias[:])

            # group output chunks so stores are fewer/larger (less SDMA
            # interference with the streaming loads)
            GROUP = 4
            out_tile = None
            for j in range(NCHUNK):
                x_tile = xp.tile([128, NK, CHUNK], mmdt)
                nc.sync.dma_start(out=x_tile[:], in_=xb[j])

                psum_tile = pp.tile([C, CHUNK], f32)
                for kt in range(NK):
                    nc.tensor.matmul(
                        psum_tile[:],
                        e_tile[:, kt, :],
                        x_tile[:, kt, :],
                        start=(kt == 0),
                        stop=(kt == NK - 1),
                    )

                if j % GROUP == 0:
                    out_tile = op.tile([C, GROUP * CHUNK], f32, tag="out")
                # bias-add on VectorE (idle; ScalarE's sequencer is the HWDGE
                # trigger engine and must not stall behind it)
                nc.vector.tensor_scalar_add(
                    out_tile[:, ts(j % GROUP, CHUNK)], psum_tile[:], bias_tile[:]
                )
                if j % GROUP == GROUP - 1:
                    nc.sync.dma_start(
                        out=outT[:, ts(j // GROUP, GROUP * CHUNK)], in_=out_tile[:]
                    )

    nc.compile()
    return nc


def _round_fp32r(a: np.ndarray) -> np.ndarray:
    """Round fp32 to e8m11 (the PE's FP32R format): round-to-nearest-even,
    low 12 mantissa bits zeroed. Returns a new contiguous fp32 array."""
    bits = np.ascontiguousarray(a, dtype=np.float32).view(np.uint32)
    rounded = (bits + 0x7FF + ((bits >> 12) & 1)) & np.uint32(0xFFFFF000)
    # keep inf/nan unmodified (inputs are finite gaussians; belt & braces)
    special = (bits & 0x7F800000) == 0x7F800000
    rounded = np.where(special, bits, rounded)
    return rounded.view(np.float32)


def _fold_E(node_0, node_1, node_2) -> np.ndarray:
    # E[(i,j,k,l), c3] = sum_{c1,c2} node_0[l,k,c1] node_1[c1,j,c2] node_2[c2,i,c3]
    E = np.einsum(
        "lkc,cjd,die->ijkle",
        node_0.astype(np.float64),
        node_1.astype(np.float64),
        node_2.astype(np.float64),
    )
    return E.reshape(K, C).astype(np.float32)


def kernel(x, node_0, node_1, node_2, bias, _trace=False, _trace_cores=None):
    x = np.asarray(x, dtype=np.float32)
    E = _fold_E(np.asarray(node_0), np.asarray(node_1), np.asarray(node_2))
    bias_np = np.asarray(bias, dtype=np.float32).reshape(C, 1)

    # blocked E: eb[p, kt, c] = E[kt*128 + p, c]
    eb = np.ascontiguousarray(E.reshape(NK, 128, C).transpose(1, 0, 2))

    if MODE not in _program_cache:
        _program_cache[MODE] = _build_program(MODE)
    nc = _program_cache[MODE]

    if MODE == "fp32r":
        eb = _round_fp32r(eb)

    in_maps = []
    for m in range(N_CORES):
        x_m = x[m * B_CORE : (m + 1) * B_CORE, :]
        # xb[j, p, kt, b] = x_m[j*CHUNK + b, kt*128 + p]
        xb_m = np.ascontiguousarray(
            x_m.reshape(NCHUNK, CHUNK, NK, 128).transpose(0, 3, 2, 1)
        )
        if MODE == "fp32r":
            xb_m = _round_fp32r(xb_m)
        in_maps.append({"xb": xb_m, "eb": eb, "bias": bias_np})

    res = run_bass_kernel_spmd(
        nc,
        in_maps,
        core_ids=list(range(N_CORES)),
        trace=_trace,
        trace_cores=_trace_cores,
    )
    results = res.results

    out = np.empty((BATCH, C), dtype=np.float32)
    for m in range(N_CORES):
        out[m * B_CORE : (m + 1) * B_CORE, :] = results[m]["outT"].T

    if _trace:
        return out, res
    return out


# revision 17
# speedup vs baseline: 1.3017x; 1.3017x over previous
"""Trainium2 Bass kernel for nn_DenseCondenser (TT contraction, 65536x4096 -> 65536x8).

The three (8,8,8) TT cores compose into a single effective matrix E (4096, 8)
(the whole map is linear in x), folded on host in float64. The device kernel
is then a memory-bound skinny matmul out = x @ E + bias, data-parallel over
the batch across 8 NeuronCores.

Device-side layout: x is staged per-core as xT (4096, 8192) so the
contraction dim lands on SBUF partitions (TensorE contracts over partitions).
Each core streams 16 chunks of (128 part, 32 ktiles, 512 batch) fp32, runs 32
accumulating matmuls per chunk with the tiny E k-tile (128, 8) as the
stationary operand, adds bias on ScalarE, and stores out.T (8, 8192).
"""

import numpy as np

import concourse.bass as bass
import concourse.mybir as mybir
import concourse.tile as tile
from concourse import bacc
from concourse.bass import ts
from concourse.bass_utils import run_bass_kernel_spmd

# Problem shapes (hardcoded per harness contract)
BATCH = 65536
K = 4096  # input features = 8**4
C = 8  # output features
N_CORES = 8
B_CORE = BATCH // N_CORES  # 8192
CHUNK = 512  # batch columns per matmul (max fp32 moving free dim)
NK = K // 128  # 32 k-tiles
NCHUNK = B_CORE // CHUNK  # 16

# "fp32" = exact (4 cyc/col), "fp32r" = fast PE mode (1 cyc/col at N>=256)
MODE = "fp32r"

_program_cache = {}


def _build_program(mode: str) -> bass.Bass:
    f32 = mybir.dt.float32
    # In fp32r mode the x/E tensors are declared float32r end-to-end; the
    # host pre-rounds them to e8m11 (what fp32r is) so HW/verifier agree.
    mmdt = mybir.dt.float32r if mode == "fp32r" else f32
    nc = bacc.Bacc(None, name="dense_condenser")

    # xb[j, p, kt, b] = x[j*CHUNK + b, kt*128 + p]: per (chunk, partition)
    # the (kt, b) payload is one contiguous 64 KiB run -> max DMA efficiency.
    xb = nc.dram_tensor("xb", (NCHUNK, 128, NK, CHUNK), mmdt, kind="ExternalInput")
    eb = nc.dram_tensor("eb", (128, NK, C), mmdt, kind="ExternalInput")
    bias = nc.dram_tensor("bias", (C, 1), f32, kind="ExternalInput")
    outT = nc.dram_tensor("outT", (C, B_CORE), f32, kind="ExternalOutput")

    with tile.TileContext(nc) as tc:
        with (
            tc.tile_pool(name="consts", bufs=1) as consts,
            tc.tile_pool(name="xp", bufs=2) as xp,
            tc.tile_pool(name="op", bufs=2) as op,
            tc.tile_pool(name="pp", bufs=2, space=bass.MemorySpace.PSUM) as pp,
        ):
            e_tile = consts.tile([128, NK, C], mmdt)
            bias_tile = consts.tile([C, 1], f32)
            nc.sync.dma_start(out=e_tile[:], in_=eb[:])
            nc.sync.dma_start(out=bias_tile[:], in_=bias[:])

            # group output chunks so stores are fewer/larger (less SDMA
            # interference with the streaming loads)
            GROUP = 4
            out_tile = None
            for j in range(NCHUNK):
                x_tile = xp.tile([128, NK, CHUNK], mmdt)
                # two half-loads: matmuls on the first half overlap the
                # second half's DMA, shrinking the end-of-stream tail
                nc.sync.dma_start(out=x_tile[:, : NK // 2], in_=xb[j, :, : NK // 2])
                nc.sync.dma_start(out=x_tile[:, NK // 2 :], in_=xb[j, :, NK // 2 :])

                psum_tile = pp.tile([C, CHUNK], f32)
                for kt in range(NK):
                    nc.tensor.matmul(
                        psum_tile[:],
                        e_tile[:, kt, :],
                        x_tile[:, kt, :],
                        start=(kt == 0),
                        stop=(kt == NK - 1),
                    )

                if j % GROUP == 0:
                    out_tile = op.tile([C, GROUP * CHUNK], f32, tag="out")
                # bias-add on VectorE (idle; ScalarE's sequencer is the HWDGE
                # trigger engine and must not stall behind it)
                nc.vector.tensor_scalar_add(
                    out_tile[:, ts(j % GROUP, CHUNK)], psum_tile[:], bias_tile[:]
                )
                if j % GROUP == GROUP - 1:
                    # stores ride the Scalar HWDGE ring, never stalling the
                    # Sync ring that feeds the streaming loads
                    nc.scalar.dma_start(
                        out=outT[:, ts(j // GROUP, GROUP * CHUNK)], in_=out_tile[:]
                    )

    nc.compile()
    return nc


def _round_fp32r(a: np.ndarray) -> np.ndarray:
    """Round fp32 to e8m11 (the PE's FP32R format): round-to-nearest-even,
    low 12 mantissa bits zeroed. Returns a new contiguous fp32 array."""
    bits = np.ascontiguousarray(a, dtype=np.float32).view(np.uint32)
    rounded = (bits + 0x7FF + ((bits >> 12) & 1)) & np.uint32(0xFFFFF000)
    # keep inf/nan unmodified (inputs are finite gaussians; belt & braces)
    special = (bits & 0x7F800000) == 0x7F800000
    rounded = np.where(special, bits, rounded)
    return rounded.view(np.float32)


def _fold_E(node_0, node_1, node_2) -> np.ndarray:
    # E[(i,j,k,l), c3] = sum_{c1,c2} node_0[l,k,c1] node_1[c1,j,c2] node_2[c2,i,c3]
    E = np.einsum(
        "lkc,cjd,die->ijkle",
        node_0.astype(np.float64),
        node_1.astype(np.float64),
        node_2.astype(np.float64),
    )
    return E.reshape(K, C).astype(np.float32)


def kernel(x, node_0, node_1, node_2, bias, _trace=False, _trace_cores=None):
    x = np.asarray(x, dtype=np.float32)
    E = _fold_E(np.asarray(node_0), np.asarray(node_1), np.asarray(node_2))
    bias_np = np.asarray(bias, dtype=np.float32).reshape(C, 1)

    # blocked E: eb[p, kt, c] = E[kt*128 + p, c]
    eb = np.ascontiguousarray(E.reshape(NK, 128, C).transpose(1, 0, 2))

    if MODE not in _program_cache:
        _program_cache[MODE] = _build_program(MODE)
    nc = _program_cache[MODE]

    if MODE == "fp32r":
        eb = _round_fp32r(eb)

    in_maps = []
    for m in range(N_CORES):
        x_m = x[m * B_CORE : (m + 1) * B_CORE, :]
        # xb[j, p, kt, b] = x_m[j*CHUNK + b, kt*128 + p]
        xb_m = np.ascontiguousarray(
            x_m.reshape(NCHUNK, CHUNK, NK, 128).transpose(0, 3, 2, 1)
        )
        if MODE == "fp32r":
            xb_m = _round_fp32r(xb_m)
        in_maps.append({"xb": xb_m, "eb": eb, "bias": bias_np})

    res = run_bass_kernel_spmd(
        nc,
        in_maps,
        core_ids=list(range(N_CORES)),
        trace=_trace,
        trace_cores=_trace_cores,
    )
    results = res.results

    out = np.empty((BATCH, C), dtype=np.float32)
    for m in range(N_CORES):
        out[m * B_CORE : (m + 1) * B_CORE, :] = results[m]["outT"].T

    if _trace:
        return out, res
    return out
